# revision 17
# baseline (speedup 1.0000x reference)
"""Trainium2 Bass kernel for nn_CalibratedNorm.

The reference module collapses algebraically to a per-(sample, channel)
affine:

    out[b,c,h,w] = x[b,c,h,w] * A[b,c] + S[b,c]

where, with gs/gsh the folded global-BN scale/shift and ms/msh the folded
mean-of-group-BNs scale/shift (all tiny [C] host math):

    alpha[b] = sigmoid( sum_c (alpha_w[c]/HW) * sum_hw x[b,c,:,:] + alpha_b )
    A[b,c]   = gs[c]  + alpha[b] * (ms[c]  - gs[c])
    S[b,c]   = gsh[c] + alpha[b] * (msh[c] - gsh[c])

Strategy: data-parallel over batch, 4 samples per core on 8 cores. The
kernel is DMA-ring bound (~360-410 GB/s of SBUF-side bytes per core when
packets are 12544B), so:

  * x travels the wire as fp16 both ways (host casts): 6.4 MB in +
    6.4 MB out per core vs 25.7 MB round trip in fp32. fp16 costs
    ~6e-4 relative error on this N(0,1) input - far inside the 2e-2
    gate.
  * The host packs each sample as rows (b*128+p) holding both channel
    halves contiguously [h=0 pixels | h=1 pixels], so every DMA runs
    at the 12544-bytes-per-partition packet size (6272B packets only
    reach ~344 GB/s; 25088B measured no better).
  * The alpha dot runs on the otherwise-idle PE as 14 accumulating
    matmuls per sample (lhsT = folded weight column, rhs = 448-pixel
    chunks of x), leaving DVE free: the 1x-rate DVE reduce_sum chain
    (3.4us per half-sample!) was V1's critical path. A 448-elem DVE
    reduce + ACT sigmoid + PE broadcast finish the gate.
  * The affine halves split across DVE (fp16 tensor_scalar) and ACT
    (Identity with scale/bias), stores chase per-sample affines.

The folded alpha weights are prescaled by 2^16 so they stay in fp16
normal range (w/HW ~ 1.6e-5 underflows fp16 normals); the sigmoid's
input scale multiplies by 2^-16 to compensate.

Measured: 46772 ns on 8 cores (vs 73428 ns fp32 baseline), rel err
6.4e-4. Breakdown: ~8.7us framework preamble + ~35.7us ring + ~2.4us
tail barrier. Variants measured worse: pair-packed 25088B rows (55.6us,
ring no faster and params late), params-first-on-HWDGE + all-DVE
affines (51.6us, ring started later and ran 331 GB/s), int8 wire+SBUF
with on-chip requant (47.7us: int8's small rows run the ring at
176 GB/s and the per-channel sums cost 3us/half on DVE/ACT at 1x).
"""

import os
import sys

import numpy as np

for _p in ("/opt/trn_rl_repo",):
    if _p not in sys.path:
        sys.path.insert(0, _p)

import concourse.bacc as bacc
import concourse.bass as bass
import concourse.tile as tile
from concourse import mybir
from concourse.bass_utils import run_bass_kernel_spmd
from concourse.tile import add_dep_helper

EPS = 1e-5
B, C, H, W, G = 32, 256, 56, 56, 32
HW = H * W  # 3136
NCORES = 8
BPC = B // NCORES  # samples per core: 4
HALVES = C // 128  # channel partition-tiles per sample: 2
ROWS = BPC * 128  # 512 rows of the per-core [ROWS, HALVES*HW] x shard
NCHUNK = 7  # alpha-dot rhs chunks per half
CHUNK = HW // NCHUNK  # 448 pixels per chunk (PSUM-bank sized)
WSCALE = 65536.0  # alpha-weight prescale to keep fp16 normal
F32 = mybir.dt.float32
F16 = mybir.dt.float16


def build_module() -> bass.Bass:
    # Bacc (not raw Bass): its compile() pass splits multi-sem waits into
    # EventSemaphore instructions — TRN2 allows at most 1 wait per
    # compute instruction and walrus codegen hard-errors otherwise.
    nc = bacc.Bacc("TRN2")

    x_in = nc.dram_tensor("x", [ROWS, HALVES * HW], F16, kind="ExternalInput")
    wq_in = nc.dram_tensor("wq", [128, HALVES], F16, kind="ExternalInput")
    tab_in = nc.dram_tensor("tab", [128, 4, HALVES], F32, kind="ExternalInput")
    ab_in = nc.dram_tensor("ab", [1, 1], F32, kind="ExternalInput")
    ones_in = nc.dram_tensor("ones", [1, 128], F32, kind="ExternalInput")
    y_out = nc.dram_tensor("out", [ROWS, HALVES * HW], F16, kind="ExternalOutput")

    with tile.TileContext(nc) as tc:
        with (
            tc.tile_pool(name="xp", bufs=BPC) as xp,
            tc.tile_pool(name="cs", bufs=1) as cs,
            tc.tile_pool(name="wk", bufs=2) as wk,
            tc.tile_pool(name="ps", bufs=BPC, space="PSUM") as ps,
            tc.tile_pool(name="pb", bufs=2, space="PSUM") as pb,
        ):
            # Tiny param tables on the SWDGE queue so they never wait
            # behind the bulk x loads on the HWDGE ring.
            # A/B measured back-to-back: params on the SWDGE (gpsimd)
            # queue beat the ACT HWDGE ring by ~2.8us (47.5 vs 50.4) —
            # the second HWDGE ring interferes with the bulk sync ring
            # more than the late param arrival costs.
            par_eng = getattr(nc, os.environ.get("KERNEL_PARAM_Q", "gpsimd"))
            wq = cs.tile([128, HALVES], F16)
            par_eng.dma_start(out=wq, in_=wq_in[:, :])
            tab = cs.tile([128, 4, HALVES], F32)
            par_eng.dma_start(out=tab, in_=tab_in[:, :, :])
            ab = cs.tile([1, 1], F32)
            par_eng.dma_start(out=ab, in_=ab_in[:, :])
            ones_row = cs.tile([1, 128], F32)
            par_eng.dma_start(out=ones_row, in_=ones_in[:, :])

            # row r = b*128 + p holds sample b, partition p, both halves:
            # channel c = h*128 + p, pixels contiguous per half.
            xv = x_in[:, :].rearrange("(b p) (h w) -> b p h w", p=128, h=HALVES)
            yv = y_out[:, :].rearrange("(b p) (h w) -> b p h w", p=128, h=HALVES)

            loads = []
            stores = []
            for b in range(BPC):
                xt = xp.tile([128, HALVES, HW], F16, name=f"xt{b}", tag="xt")
                # One 1.6MB DMA per sample: 12544B per partition row.
                loads.append(nc.sync.dma_start(out=xt[:, :, :], in_=xv[b]))

                # alpha dot on PE: zp[0,n] accumulates
                #   sum_h sum_k sum_p wq[p,h] * x[p,h,448k+n]
                zp = ps.tile([1, CHUNK], F32, name=f"zp{b}", tag="zp")
                for h in range(HALVES):
                    for k in range(NCHUNK):
                        nc.tensor.matmul(
                            zp[:, :],
                            lhsT=wq[:, h : h + 1],
                            rhs=xt[:, h, k * CHUNK : (k + 1) * CHUNK],
                            start=(h == 0 and k == 0),
                            stop=(h == HALVES - 1 and k == NCHUNK - 1),
                        )
                # z = sum_n zp[0,n]; alpha = sigmoid(z/WSCALE + alpha_b)
                zs = wk.tile([1, 1], F32, name=f"zs{b}", tag="zs")
                nc.vector.reduce_sum(
                    out=zs, in_=zp[:, :], axis=mybir.AxisListType.X
                )
                al = wk.tile([1, 1], F32, name=f"al{b}", tag="al")
                nc.scalar.activation(
                    out=al, in_=zs[:, :],
                    func=mybir.ActivationFunctionType.Sigmoid,
                    bias=ab[0:1, 0:1], scale=float(1.0 / WSCALE),
                )
                # broadcast alpha to all partitions, move to SBUF
                bc = pb.tile([128, 1], F32, name=f"bc{b}", tag="bc")
                nc.tensor.matmul(
                    bc[:, :], lhsT=ones_row[:, :], rhs=al[:, :],
                    start=True, stop=True,
                )
                ac = wk.tile([128, 1], F32, name=f"ac{b}", tag="ac")
                nc.vector.tensor_copy(out=ac, in_=bc[:, :])

                # A = gs + alpha*dms ; S = gsh + alpha*dmsh   [128, 2]
                A = wk.tile([128, HALVES], F32, name=f"A{b}", tag="A")
                Sh = wk.tile([128, HALVES], F32, name=f"S{b}", tag="S")
                nc.vector.tensor_scalar_mul(out=A, in0=tab[:, 1, :], scalar1=ac)
                nc.vector.tensor_add(out=A, in0=A[:, :], in1=tab[:, 0, :])
                nc.vector.tensor_scalar_mul(out=Sh, in0=tab[:, 3, :], scalar1=ac)
                nc.vector.tensor_add(out=Sh, in0=Sh[:, :], in1=tab[:, 2, :])

                # Fused affine on DVE (fp16 tensor_scalar, fp32 scalars),
                # one op per half; single 1.6MB store per sample.
                for h in range(HALVES):
                    nc.vector.tensor_scalar(
                        out=xt[:, h, :], in0=xt[:, h, :],
                        scalar1=A[:, h : h + 1], scalar2=Sh[:, h : h + 1],
                        op0=mybir.AluOpType.mult, op1=mybir.AluOpType.add,
                    )
                stores.append(nc.sync.dma_start(out=yv[b], in_=xt[:, :, :]))

            # Ring order L0 L1 L2 S0 L3 S1 S2 S3: ordering-only edges (no
            # sems) keep stores behind the loads that gate later alpha
            # chains, but let sample 0's store slot in before the last
            # load so the ring never idles across the loads->stores
            # transition (a ~3us restart gap in the strict loads-first
            # order). The PE-dot alpha chain is short enough that L3
            # landing ~4us later still leaves sample 3's store ~4us of
            # ring-margin.
            add_dep_helper(
                stores[0].ins, loads[2].ins, sync=False,
                reason="S0 after L2 on SP ring",
            )
            add_dep_helper(
                loads[3].ins, stores[0].ins, sync=False,
                reason="L3 right after S0 on SP ring",
            )
            for st in stores[1:]:
                add_dep_helper(
                    st.ins, loads[-1].ins, sync=False,
                    reason="remaining loads drain before stores",
                )

    nc.compile()
    return nc


_NC_CACHE: list = []


def _get_module() -> bass.Bass:
    if not _NC_CACHE:
        _NC_CACHE.append(build_module())
    return _NC_CACHE[0]


def _prep_in_maps(inputs: dict) -> list[dict]:
    x = np.asarray(inputs["x"], dtype=np.float32)
    alpha_w = np.asarray(inputs["alpha_w"], dtype=np.float32)
    alpha_b = np.asarray(inputs["alpha_b"], dtype=np.float32)
    g_w = np.asarray(inputs["g_w"], dtype=np.float32)
    g_b = np.asarray(inputs["g_b"], dtype=np.float32)
    g_rm = np.asarray(inputs["g_rm"], dtype=np.float32)
    g_rv = np.asarray(inputs["g_rv"], dtype=np.float32)
    grp_w = np.asarray(inputs["grp_w"], dtype=np.float32)
    grp_b = np.asarray(inputs["grp_b"], dtype=np.float32)
    grp_rm = np.asarray(inputs["grp_rm"], dtype=np.float32)
    grp_rv = np.asarray(inputs["grp_rv"], dtype=np.float32)

    gs = g_w / np.sqrt(g_rv + EPS)
    gsh = g_b - g_rm * gs
    sg = grp_w / np.sqrt(grp_rv + EPS)  # [G, C]
    ms = sg.mean(axis=0)
    msh = (grp_b - grp_rm * sg).mean(axis=0)
    dms = ms - gs
    dmsh = msh - gsh

    ch = (np.arange(HALVES)[None, :] * 128 + np.arange(128)[:, None])  # [128, HALVES]
    tab = np.empty((128, 4, HALVES), dtype=np.float32)
    tab[:, 0, :] = gs[ch]
    tab[:, 1, :] = dms[ch]
    tab[:, 2, :] = gsh[ch]
    tab[:, 3, :] = dmsh[ch]

    wq = ((alpha_w * (WSCALE / HW))[ch]).astype(np.float16)  # [128, HALVES]
    ab = np.array([[alpha_b.reshape(-1)[0]]], dtype=np.float32)
    ones = np.ones((1, 128), dtype=np.float32)

    # Pack rows as (b*128+p) -> [h=0 pixels | h=1 pixels], fp16.
    x16 = x.astype(np.float16)  # [B, C, H, W]
    in_maps = []
    for k in range(NCORES):
        xs = (
            x16[k * BPC : (k + 1) * BPC]
            .reshape(BPC, HALVES, 128, HW)
            .transpose(0, 2, 1, 3)
            .reshape(ROWS, HALVES * HW)
        )
        in_maps.append(
            {"x": np.ascontiguousarray(xs), "wq": wq, "tab": tab, "ab": ab,
             "ones": ones}
        )
    return in_maps


def _unpack_core_out(raw: np.ndarray) -> np.ndarray:
    """Device [ROWS, HALVES*HW] (packed rows) -> [BPC, C, H, W] fp32."""
    return (
        np.asarray(raw)
        .astype(np.float32)
        .reshape(BPC, 128, HALVES, HW)
        .transpose(0, 2, 1, 3)
        .reshape(BPC, C, H, W)
    )


def _run(inputs: dict, trace: bool = False, trace_cores=None):
    nc = _get_module()
    in_maps = _prep_in_maps(inputs)
    res = run_bass_kernel_spmd(
        nc, in_maps, core_ids=list(range(NCORES)), trace=trace,
        trace_cores=trace_cores,
    )
    outs = [_unpack_core_out(r["out"]) for r in res.results]
    full = np.concatenate(outs, axis=0)
    return full, res


def kernel(**inputs) -> np.ndarray:
    out, _ = _run(inputs, trace=False)
    return out
